# revision 19
# baseline (speedup 1.0000x reference)
"""2D Haar DWT (analysis) on 8 Trainium2 NeuronCores.

Input  x: (16, 64, 256, 256) f32  -> 1024 independent 256x256 images.
Output: tuple (LL, LH, HL, HH), each (16, 64, 128, 128) f32.

With Haar filters the DWT is a 2x2 butterfly: for each 2x2 block
(a b / c d), with the 0.5 scale folded into a host-side prescale:
    LL = a+b+c+d, LH = a-b+c-d, HL = a+b-c-d, HH = a-b-c+d
i.e. two levels of adds/subs -- no matmul. fp32 matmuls stream at half
rate on the PE and would dominate (measured 505us); plain VectorE adds
finish in ~145us per core, under the ~160us DMA-fabric floor for 67MB
of HBM traffic, so the kernel runs at the DMA roofline.

Layout (everything unit-stride, partition dim = image index):
  - host prescales x by 0.5 and deinterleaves even/odd columns
  - per core 128 images; rows processed in chunks; each chunk is one
    fully contiguous DRAM block [img, hc rows] so DMA descriptors are
    maximal (2MB transfers, 16KB/partition runs)
  - per chunk: one input DMA, 6 VectorE tensor ops, one output DMA.

Measured on hardware (neuron-profile, core 0): ~169-171us fast mode
(~198us when the DMA-engine-15 straggler fires), vs ~187us naive HBM
roofline at 358 GB/s/core and 505us for the matmul formulation.
"""

import numpy as np

import concourse.bacc as bacc
import concourse.tile as tile
from concourse import mybir
from concourse.bass_utils import run_bass_kernel_spmd

N_CORES = 8
B, C, H, W = 16, 64, 256, 256
N_IMG = B * C                    # 1024
P = N_IMG // N_CORES             # 128 images per core = partition dim
Wh = W // 2                      # 128
HC_BIG, N_BIG = 16, 16           # 16 chunks of 16 rows (2MB transfers)
HC_SM, N_SM = 4, 0               # (small tail chunks measured no faster)
assert HC_BIG * N_BIG + HC_SM * N_SM == H
F32 = mybir.dt.float32

_CACHE = {}


def _butterfly(nc, xt, mid, op, hc):
    """Emit the 6 VectorE ops for one chunk; returns the output tile."""
    xv = xt.rearrange("p (h e w) -> p h e w", h=hc, e=2, w=Wh)
    xe = xv[:, :, 0, :].rearrange("p (i f) w -> p i f w", f=2)
    xo = xv[:, :, 1, :].rearrange("p (i f) w -> p i f w", f=2)
    sw = mid.tile([P, hc // 2, 2, Wh], F32, tag="sw")
    dw = mid.tile([P, hc // 2, 2, Wh], F32, tag="dw")
    nc.vector.tensor_add(sw, xe, xo)
    nc.vector.tensor_sub(dw, xe, xo)
    ot = op.tile([P, 4 * (hc // 2) * Wh], F32, tag="ot")
    ov = ot.rearrange("p (b i w) -> p b i w", b=4, i=hc // 2, w=Wh)
    nc.vector.tensor_add(ov[:, 0], sw[:, :, 0, :], sw[:, :, 1, :])  # LL
    nc.vector.tensor_add(ov[:, 1], dw[:, :, 0, :], dw[:, :, 1, :])  # LH
    nc.vector.tensor_sub(ov[:, 2], sw[:, :, 0, :], sw[:, :, 1, :])  # HL
    nc.vector.tensor_sub(ov[:, 3], dw[:, :, 0, :], dw[:, :, 1, :])  # HH
    return ot


def _build_program():
    nc = bacc.Bacc(
        "TRN2",
        target_bir_lowering=False,
        debug=False,
        enable_asserts=False,
        num_devices=N_CORES,
    )
    xb = nc.dram_tensor("xb", [N_BIG, P, HC_BIG * W], F32, kind="ExternalInput").ap()
    xs = (nc.dram_tensor("xs", [N_SM, P, HC_SM * W], F32, kind="ExternalInput").ap()
          if N_SM else None)
    ob = nc.dram_tensor("ob", [N_BIG, P, HC_BIG * W], F32, kind="ExternalOutput").ap()
    os_ = (nc.dram_tensor("os", [N_SM, P, HC_SM * W], F32, kind="ExternalOutput").ap()
           if N_SM else None)

    with tile.TileContext(nc) as tc:
        with (
            tc.tile_pool(name="xp", bufs=5) as xp,
            tc.tile_pool(name="mid", bufs=3) as mid,
            tc.tile_pool(name="op", bufs=4) as op,
        ):
            for k in range(N_BIG):
                xt = xp.tile([P, HC_BIG * W], F32, tag="xt")
                nc.sync.dma_start(out=xt, in_=xb[k])
                ot = _butterfly(nc, xt, mid, op, HC_BIG)
                nc.scalar.dma_start(out=ob[k], in_=ot)
            for k in range(N_SM):
                xt = xp.tile([P, HC_SM * W], F32, tag="xt")
                nc.sync.dma_start(out=xt, in_=xs[k])
                ot = _butterfly(nc, xt, mid, op, HC_SM)
                nc.scalar.dma_start(out=os_[k], in_=ot)
    nc.compile()
    return nc


def kernel(x, m_l0, m_l1, m_h0, m_h1):
    x = np.asarray(x, dtype=np.float32)
    assert x.shape == (B, C, H, W), x.shape

    if "nc" not in _CACHE:
        _CACHE["nc"] = _build_program()
    nc = _CACHE["nc"]

    # prescale by 0.5 (exact) and split even/odd columns: [N, H, 2, W/2]
    xsp = (x.reshape(N_IMG, H, W // 2, 2) * np.float32(0.5)).transpose(0, 1, 3, 2)
    r_split = N_BIG * HC_BIG
    in_maps = []
    for s in range(N_CORES):
        shard = xsp[s * P:(s + 1) * P]  # [128, 256, 2, 128]
        big = shard[:, :r_split].reshape(P, N_BIG, HC_BIG * W).transpose(1, 0, 2)
        sm = shard[:, r_split:].reshape(P, N_SM, HC_SM * W).transpose(1, 0, 2)
        in_maps.append({
            "xb": np.ascontiguousarray(big),
            "xs": np.ascontiguousarray(sm),
        })

    res = run_bass_kernel_spmd(nc, in_maps, core_ids=list(range(N_CORES)))

    parts = []
    for s in range(N_CORES):
        img = np.empty((P, 4, H // 2, Wh), np.float32)
        obig = res.results[s]["ob"].reshape(N_BIG, P, 4, HC_BIG // 2, Wh)
        img[:, :, :r_split // 2] = obig.transpose(1, 2, 0, 3, 4).reshape(
            P, 4, r_split // 2, Wh)
        if N_SM:
            osm = res.results[s]["os"].reshape(N_SM, P, 4, HC_SM // 2, Wh)
            img[:, :, r_split // 2:] = osm.transpose(1, 2, 0, 3, 4).reshape(
                P, 4, (H - r_split) // 2, Wh)
        parts.append(img)
    full = np.concatenate(parts, axis=0).reshape(B, C, 4, H // 2, Wh)
    LL = np.ascontiguousarray(full[:, :, 0])
    LH = np.ascontiguousarray(full[:, :, 1])
    HL = np.ascontiguousarray(full[:, :, 2])
    HH = np.ascontiguousarray(full[:, :, 3])
    return (LL, LH, HL, HH)
